# revision 7
# baseline (speedup 1.0000x reference)
"""Causal self-attention (B=2048, T=128, C=192, H=6, D=32) on 8 TRN2 cores.

Data-parallel over batch: 256 elems/core. Per elem, everything stays on-chip:
  x -> (cast bf16, DMA-xbar transpose) xT -> qkv matmuls (bias via K=1 ones
  matmul) -> q,k DMA-transposed to [d,t]; S^ = QK^T per head (row-tiled PE,
  scale prefolded into w_q); causal mask added via an accumulating matmul
  (-1e10 * strict-upper as lhsT, identity rhs); exp on ScalarE with fused
  per-head rowsums; normalize P on VectorE (per-partition scalars);
  P transposed via DMA-xbar; y^T = V^T @ P^T (col-tiled PE); proj with bias
  via ones-row in the K dimension.
"""

import sys

sys.path.insert(0, "/opt/trn_rl_repo")

import numpy as np
import ml_dtypes

N_CORES = 8
B, T, C = 2048, 128, 192
NH, HD = 6, 32
BL = B // N_CORES  # 256 per core

_CACHE = {}


def _build(bl):
    from contextlib import ExitStack

    import concourse.bass as bass
    import concourse.mybir as mybir
    import concourse.tile as tile
    from concourse import bacc

    fp32 = mybir.dt.float32
    bf16 = mybir.dt.bfloat16
    AF = mybir.ActivationFunctionType
    ALU = mybir.AluOpType

    nc = bacc.Bacc("TRN2", target_bir_lowering=False, debug=False)

    x_d = nc.dram_tensor("x", [bl, T, C], fp32, kind="ExternalInput")
    wA_d = nc.dram_tensor("wA", [128, 576], bf16, kind="ExternalInput")
    wB_d = nc.dram_tensor("wB", [64, 576], bf16, kind="ExternalInput")
    bq_d = nc.dram_tensor("bq", [1, 576], bf16, kind="ExternalInput")
    wpA_d = nc.dram_tensor("wpA", [128, 192], bf16, kind="ExternalInput")
    wpB_d = nc.dram_tensor("wpB", [65, 192], bf16, kind="ExternalInput")
    mask_d = nc.dram_tensor("maskA", [128, 128], bf16, kind="ExternalInput")
    ident_d = nc.dram_tensor("ident", [128, 128], bf16, kind="ExternalInput")
    out_d = nc.dram_tensor("out", [bl, T, C], fp32, kind="ExternalOutput")

    with tile.TileContext(nc) as tc, ExitStack() as ctx:
        consts = ctx.enter_context(tc.tile_pool(name="consts", bufs=1))
        sb = ctx.enter_context(tc.tile_pool(name="sb", bufs=3))
        ps = ctx.enter_context(
            tc.tile_pool(name="ps", bufs=1, space=bass.MemorySpace.PSUM)
        )

        # --- constants ---
        wA = consts.tile([128, 576], bf16)
        nc.sync.dma_start(wA[:], wA_d[:])
        wB = consts.tile([64, 576], bf16)
        nc.sync.dma_start(wB[:], wB_d[:])
        bq = consts.tile([1, 576], bf16)
        nc.sync.dma_start(bq[:], bq_d[:])
        wpA = consts.tile([128, 192], bf16)
        nc.sync.dma_start(wpA[:], wpA_d[:])
        wpB = consts.tile([65, 192], bf16)
        nc.sync.dma_start(wpB[:], wpB_d[:])

        ones1 = consts.tile([1, 128], bf16)
        nc.gpsimd.memset(ones1[:], 1.0)

        # maskA[k, t] = -1e10 where k > t else 0  (used as lhsT: adds -1e10
        # to S[t, s] for s > t); ident is the rhs of the mask matmul
        maskA = consts.tile([128, 128], bf16)
        nc.sync.dma_start(maskA[:], mask_d[:])
        ident = consts.tile([128, 128], bf16)
        nc.sync.dma_start(ident[:], ident_d[:])

        # PSUM: 8 banks as 8 single-buf tags. qkv reuses A/B/C with S3/4/5.
        def pt(tag, shape):
            return ps.tile(shape, fp32, tag=tag, name=f"ps_{tag}")

        for b in range(bl):
            xf = sb.tile([128, 192], fp32, tag="xf")
            nc.sync.dma_start(xf[:], x_d[b])
            x16 = sb.tile([128, 256], bf16, tag="x16")
            nc.vector.tensor_copy(x16[:, 0:192], xf[:])
            xT1 = sb.tile([128, 128], bf16, tag="xT1")
            xT2 = sb.tile([128, 128], bf16, tag="xT2")
            nc.sync.dma_start_transpose(xT1[:], x16[:, 0:128])
            nc.sync.dma_start_transpose(xT2[:], x16[:, 128:256])

            qps = pt("A", [128, 192])
            kps = pt("B", [128, 192])
            vps = pt("C", [128, 192])
            for gi, g in enumerate([qps, kps, vps]):
                c0 = gi * 192
                nc.tensor.matmul(
                    g[:], ones1[:], bq[:, c0 : c0 + 192], start=True, stop=False
                )
                nc.tensor.matmul(
                    g[:], xT1[:], wA[:, c0 : c0 + 192], start=False, stop=False
                )
                nc.tensor.matmul(
                    g[:], xT2[0:64, :], wB[:, c0 : c0 + 192], start=False, stop=True
                )

            qk16 = sb.tile([128, 448], bf16, tag="qk16")
            v16 = sb.tile([128, 192], bf16, tag="v16")
            nc.vector.tensor_copy(qk16[:, 0:192], qps[:])
            nc.vector.tensor_copy(qk16[:, 192:384], kps[:])
            nc.scalar.copy(v16[:], vps[:])

            qT1 = sb.tile([128, 128], bf16, tag="qT1")
            qT2 = sb.tile([128, 128], bf16, tag="qT2")
            kT1 = sb.tile([128, 128], bf16, tag="kT1")
            kT2 = sb.tile([128, 128], bf16, tag="kT2")
            nc.sync.dma_start_transpose(qT1[:], qk16[:, 0:128])
            nc.sync.dma_start_transpose(qT2[:], qk16[:, 128:256])
            nc.sync.dma_start_transpose(kT1[:], qk16[:, 192:320])
            nc.sync.dma_start_transpose(kT2[:], qk16[:, 320:448])

            # S_h[t, s] in its own PSUM bank; mask first, then QK^T
            stags = ["D", "E", "F", "A", "B", "C"]
            S = [pt(stags[h], [128, 128]) for h in range(NH)]
            for h in range(NH):
                nc.tensor.matmul(S[h][:], maskA[:], ident[:], start=True, stop=False)
            for h in range(NH):
                qT = qT1 if h < 4 else qT2
                kT = kT1 if h < 4 else kT2
                r = (h % 4) * 32
                nc.tensor.matmul(
                    S[h][:],
                    qT[r : r + 32, :],
                    kT[r : r + 32, :],
                    start=False,
                    stop=True,
                    tile_position=(r, 0),
                )

            rsum = sb.tile([128, 8], fp32, tag="rsum")
            P16 = sb.tile([128, 6, 128], bf16, tag="P16")
            for h in range(NH):
                nc.scalar.activation(
                    P16[:, h, :],
                    S[h][:],
                    AF.Exp,
                    accum_out=rsum[:, h : h + 1],
                )
            rrec = sb.tile([128, 8], fp32, tag="rrec")
            nc.vector.reciprocal(rrec[:, 0:6], rsum[:, 0:6])
            Pn = sb.tile([128, 6, 128], bf16, tag="Pn")
            for h in range(NH):
                nc.vector.tensor_scalar_mul(
                    Pn[:, h, :], P16[:, h, :], rrec[:, h : h + 1]
                )
            PT = sb.tile([128, 6, 128], bf16, tag="PT")
            for h in range(NH):
                nc.sync.dma_start_transpose(PT[:, h, :], Pn[:, h, :])

            # y^T: col-tiled, heads stacked on partitions
            yt = pt("G", [128, 2, 128])
            for h in range(NH):
                r = (h % 4) * 32
                j = 0 if h < 4 else 1
                nc.tensor.matmul(
                    yt[r : r + 32, j, :],
                    v16[:, h * 32 : h * 32 + 32],
                    PT[:, h, :],
                    start=True,
                    stop=True,
                    tile_position=(0, r),
                )

            yTa = sb.tile([128, 128], bf16, tag="yTa")
            yTb = sb.tile([65, 128], bf16, tag="yTb")
            nc.vector.tensor_copy(yTa[:], yt[:, 0, :])
            nc.vector.tensor_copy(yTb[0:64, :], yt[0:64, 1, :])
            nc.gpsimd.memset(yTb[64:65, :], 1.0)

            outp = pt("H", [128, 192])
            nc.tensor.matmul(outp[:], yTa[:], wpA[:], start=True, stop=False)
            nc.tensor.matmul(outp[:], yTb[:], wpB[:], start=False, stop=True)
            outs = sb.tile([128, 192], fp32, tag="outs")
            nc.scalar.copy(outs[:], outp[:])
            nc.sync.dma_start(out_d[b], outs[:])

    nc.finalize()
    return nc


def _prep_inputs(x, w_qkv, b_qkv, w_proj, b_proj, bl):
    bf = ml_dtypes.bfloat16
    scale = 1.0 / np.sqrt(HD)
    w2 = np.array(w_qkv, dtype=np.float32, copy=True)
    b2 = np.array(b_qkv, dtype=np.float32, copy=True)
    w2[:, 0:C] *= scale
    b2[0:C] *= scale
    wA = w2[0:128].astype(bf)
    wB = w2[128:192].astype(bf)
    bq = b2[None, :].astype(bf)
    wpA = np.asarray(w_proj)[0:128].astype(bf)
    wpB = np.concatenate(
        [np.asarray(w_proj)[128:192], np.asarray(b_proj)[None, :]], axis=0
    ).astype(bf)
    maskA = np.tril(np.full((128, 128), -1e10, np.float32), -1).astype(bf)
    ident = np.eye(128, dtype=np.float32).astype(bf)
    xs = np.ascontiguousarray(np.asarray(x, dtype=np.float32)).reshape(
        -1, bl, T, C
    )
    maps = []
    for i in range(xs.shape[0]):
        maps.append(
            {
                "x": xs[i],
                "wA": wA,
                "wB": wB,
                "bq": bq,
                "wpA": wpA,
                "wpB": wpB,
                "maskA": maskA,
                "ident": ident,
            }
        )
    return maps


def _run(x, w_qkv, b_qkv, w_proj, b_proj, bl=BL, n_cores=N_CORES, trace=False):
    from concourse.bass_utils import run_bass_kernel_spmd

    key = bl
    if key not in _CACHE:
        _CACHE[key] = _build(bl)
    nc = _CACHE[key]
    maps = _prep_inputs(x, w_qkv, b_qkv, w_proj, b_proj, bl)[:n_cores]
    res = run_bass_kernel_spmd(
        nc, maps, core_ids=list(range(len(maps))), trace=trace
    )
    out = np.concatenate([r["out"] for r in res.results], axis=0)
    return out, res


def kernel(x, w_qkv, b_qkv, w_proj, b_proj):
    out, _ = _run(x, w_qkv, b_qkv, w_proj, b_proj)
    return out.reshape(B, T, C).astype(np.float32)
